# revision 20
# baseline (speedup 1.0000x reference)
"""Binary-weight 3x3 SAME conv + bias + ReLU for (16,224,224,64)x(3,3,64,128),
distributed over 8 Trainium2 NeuronCores.

Distribution: data-parallel over the batch (N=16 -> 2 images per core), conv
weights (tiny, binarized to +/-1) replicated on every core. Forward only, so
no collectives are needed.

Per-core kernel design:
  - Host prep: x is transposed to NCHW and zero-padded by a 1-pixel halo into
    an fp16 tensor xp[2, 64, 226, 226]; W is binarized with sign() and packed
    per-tap into an fp16 [128, 9*128] lhsT block (rows 0-63 and 64-127 are
    identical copies, one per image slot).
  - SBUF layout puts input channels on partitions: image 0 occupies
    partitions 0-63, image 1 partitions 64-127. The two images' K=64 matmuls
    auto-derive PE tile_position (0,0)/(64,0) and run concurrently on
    disjoint row-groups of the 128x128 PE array - full array utilization
    without stacking copies.
  - A 3x3 conv is 9 shifted matmuls accumulated in one PSUM bank. The input
    strip is flattened [c, h*226]; a +/-1 column shift is a free-dim AP
    offset, a row shift is +/-226. Each PSUM tile covers two padded output
    rows (N = 2*226-2 = 450 <= 512 fp32 bank limit); the pad columns between
    the rows compute garbage that is simply never drained. Measured steady
    duo cadence is N/2.4GHz + 2.5ns = ~190ns, 98.7% of PE peak.
  - Drain = bias-add + ReLU straight out of PSUM, alternating between
    ScalarE (activation Relu with per-partition bias) and VectorE
    (tensor_scalar add/max) to split the elementwise load, writing dense
    fp32 [cout, 16*224] staging tiles that DMA contiguously to HBM.
  - fp16 activations keep the absmax error ~2e-4 relative to the fp32
    reference (weights are exactly +/-1 in fp16, accumulation is fp32 PSUM).
"""

import os
import sys

import numpy as np

for _p in ("/opt/trn_rl_repo", "/root/.axon_site/_ro/trn_rl_repo", "/root/.axon_site"):
    if os.path.isdir(_p) and _p not in sys.path:
        sys.path.append(_p)

import concourse.bass as bass
import concourse.mybir as mybir
import concourse.tile as tile
from concourse import bacc
from concourse.bass_utils import run_bass_kernel_spmd

# Problem shape (hardcoded per contract).
N_FULL, H, W_, CIN, COUT = 16, 224, 224, 64, 128
N_CORES = 8
IMGS = N_FULL // N_CORES  # images per core


def build_conv_program(imgs=IMGS, h=H, w=W_, cin=CIN, cout=COUT, strip_out=16):
    """Build the per-core Bass program. Parameterized so a scaled-down
    variant can be validated in CoreSim."""
    assert imgs == 2 and cin == 64 and cout == 128
    assert h % strip_out == 0 and strip_out % 2 == 0
    hp, wp = h + 2, w + 2
    n_strips = h // strip_out
    pairs = strip_out // 2  # output-row pairs per strip
    rows = strip_out + 2  # input rows needed per strip
    nfree = 2 * wp - 2  # matmul free dim (2 padded rows, minus trailing pads)
    npsum = 2 * wp  # PSUM tile width (trailing 2 cols never written/read)
    assert nfree <= 512
    f16, f32 = mybir.dt.float16, mybir.dt.float32

    nc = bacc.Bacc("TRN2", target_bir_lowering=False, debug=False)
    xp = nc.dram_tensor("xp", [imgs, cin, hp, wp], f16, kind="ExternalInput")
    wq = nc.dram_tensor("wq", [2 * cin, 9 * cout], f16, kind="ExternalInput")
    bias = nc.dram_tensor("bias", [cout, 1], f32, kind="ExternalInput")
    out = nc.dram_tensor("out", [imgs, cout, h, w], f32, kind="ExternalOutput")

    with tile.TileContext(nc) as tc:
        with (
            tc.tile_pool(name="const", bufs=1) as cpool,
            tc.tile_pool(name="xin", bufs=3) as xpool,
            tc.tile_pool(name="ps", bufs=8, space="PSUM") as pspool,
            tc.tile_pool(name="ostage", bufs=4) as opool,
        ):
            wsb = cpool.tile([128, 9 * cout], f16)
            nc.sync.dma_start(out=wsb[:], in_=wq[:])
            bsb = cpool.tile([cout, 1], f32)
            nc.sync.dma_start(out=bsb[:], in_=bias[:])
            warm = cpool.tile([cout, 1], f32)
            # PE warm-up: a dense ~9us accumulation group of dummy matmuls
    # on memset-zero SBUF (no DMA dependency) trips the HAM busy
            # window (~3.4us) early, so the real stream starts at 2.4GHz
            # instead of running its first ~2 strips at 1.2GHz. A single
            # accumulation group is essential: independent start/stop
            # matmuls on one tile get WAW-serialized by semaphores and
            # the resulting gappy burst never un-throttles the clock.
            dummy_x = cpool.tile([128, 512], f16)
            nc.vector.memset(dummy_x[:], 0)
            dummy_ps = pspool.tile([cout, 512], f32, name="dummy_ps", tag="pt")
            n_warm = 30
            for k in range(n_warm):
                nc.tensor.matmul(
                    dummy_ps[:],
                    dummy_x[:, 0:cout],
                    dummy_x[:, 0:512],
                    start=(k == 0),
                    stop=(k == n_warm - 1),
                )

            for s in range(n_strips):
                r0 = s * strip_out
                # Input strip: both images' channels stacked on partitions;
                # free dim = flattened padded rows r0 .. r0+rows-1.
                xt = xpool.tile([128, rows * wp], f16)
                # Half-strip loads (on the scalar engine's HWDGE queue,
                # away from bulk output traffic on sync) so the first duos
                # of a strip unblock after ~half the load time.
                rh0 = rows // 2
                for ra, rb in [(0, rh0), (rh0, rows)]:
                    nc.scalar.dma_start(
                        out=xt[:, ra * wp : rb * wp],
                        in_=xp[:][:, :, r0 + ra : r0 + rb, :].rearrange(
                            "i c r q -> (i c) (r q)"
                        ),
                    )
                if s == 0:
                    # Warm the ACT Relu spline table (~2.7us one-time load,
                    # unmodeled by the scheduler) AFTER the first strip's
                    # load triggers - ahead of them on the scalar stream it
                    # stalls the whole input pipe behind the table DMA.
                    nc.scalar.activation(
                        warm[:], bsb[:], mybir.ActivationFunctionType.Relu, bias=0.0
                    )

                ots = [
                    opool.tile([cout, strip_out * w], f32, name=f"ot{i}", tag=f"ot{i}")
                    for i in range(imgs)
                ]

                # Per-duo PSUM flow: 2 banks allocated per output-row pair,
                # 8-slot pool lets ~4 duos pipeline; drains release slots in
                # issue order so the matmul stream never hits a block-sized
                # PSUM barrier.
                for d in range(pairs):
                    pt = [
                        pspool.tile([cout, npsum], f32, name=f"pt_{s}_{d}_{i}", tag="pt")
                        for i in range(imgs)
                    ]
                    # 9 shifted taps accumulate; emission order (t, i)
                    # alternates images so adjacent PE matmuls land on
                    # disjoint row-groups and run concurrently.
                    for t in range(9):
                        dy, dx = divmod(t, 3)
                        base = (2 * d + dy) * wp + dx
                        for i in range(imgs):
                            nc.tensor.matmul(
                                pt[i][:, :nfree],
                                wsb[i * cin : (i + 1) * cin, t * cout : (t + 1) * cout],
                                xt[i * cin : (i + 1) * cin, base : base + nfree],
                                start=(t == 0),
                                stop=(t == 8),
                            )
                    # Drain: bias + ReLU, skipping the 2 pad columns per row.
                    for i in range(imgs):
                        src = pt[i][:].rearrange("p (r q) -> p r q", q=wp)[:, :, :w]
                        dst = ots[i][:, d * 2 * w : (d + 1) * 2 * w].rearrange(
                            "p (r q) -> p r q", q=w
                        )
                        if (d + i) % 2 == 0:
                            nc.scalar.activation(
                                dst,
                                src,
                                mybir.ActivationFunctionType.Relu,
                                bias=bsb[:, 0:1],
                            )
                        else:
                            nc.vector.tensor_scalar(
                                dst,
                                src,
                                bsb[:, 0:1],
                                0.0,
                                mybir.AluOpType.add,
                                mybir.AluOpType.max,
                            )
                    # Half-strip output DMA on the otherwise-idle sync
                    # engine; quarter-strip on the final strip so the
                    # kernel-tail transfer is short. (Finer everywhere
                    # regresses: the extra DMA-completion stalls break the
                    # PE busy window and re-throttle the HAM clock.)
                    qg = max(1, pairs // (4 if s == n_strips - 1 else 2))
                    if d % qg == qg - 1:
                        quart = d // qg
                        rh = 2 * qg
                        for i in range(imgs):
                            nc.sync.dma_start(
                                out=out[:][
                                    i, :, r0 + quart * rh : r0 + (quart + 1) * rh, :
                                ].rearrange("c r q -> c (r q)"),
                                in_=ots[i][:, quart * rh * w : (quart + 1) * rh * w],
                            )

    nc.compile()
    return nc


def prep_inputs(x, W, b, imgs=IMGS, h=H, w=W_, cin=CIN, cout=COUT, n_cores=N_CORES):
    """Host-side shard + layout prep. Returns per-core input maps."""
    hp, wp = h + 2, w + 2
    n = imgs * n_cores
    # Binarize weights; pack per-tap lhsT blocks, duplicated per image slot.
    wq_np = np.sign(np.asarray(W, dtype=np.float32)).astype(np.float16)
    wq_host = np.empty((2 * cin, 9 * cout), np.float16)
    for t in range(9):
        dy, dx = divmod(t, 3)
        wq_host[0:cin, t * cout : (t + 1) * cout] = wq_np[dy, dx]
        wq_host[cin : 2 * cin, t * cout : (t + 1) * cout] = wq_np[dy, dx]
    bias_host = np.ascontiguousarray(
        np.asarray(b, dtype=np.float32).reshape(cout, 1)
    )
    # NHWC -> NCHW, fp16, 1-pixel zero halo.
    xp_host = np.zeros((n, cin, hp, wp), np.float16)
    xp_host[:, :, 1 : h + 1, 1 : w + 1] = np.asarray(x).transpose(0, 3, 1, 2)
    return [
        {
            "xp": np.ascontiguousarray(xp_host[c * imgs : (c + 1) * imgs]),
            "wq": wq_host,
            "bias": bias_host,
        }
        for c in range(n_cores)
    ]


_NC_CACHE = {}


def _get_program():
    if "nc" not in _NC_CACHE:
        _NC_CACHE["nc"] = build_conv_program()
    return _NC_CACHE["nc"]


def kernel(x, W, b):
    """Full-input entry point: x (16,224,224,64) f32 NHWC, W (3,3,64,128) f32
    HWIO, b (128,) f32 -> (16,224,224,128) f32 NHWC."""
    nc = _get_program()
    in_maps = prep_inputs(x, W, b)
    res = run_bass_kernel_spmd(nc, in_maps, core_ids=list(range(N_CORES)))
    # Gather: per-core [2, 128, 224, 224] -> full NHWC.
    full = np.empty((N_FULL, H, W_, COUT), np.float32)
    for c in range(N_CORES):
        o = res.results[c]["out"]
        full[c * IMGS : (c + 1) * IMGS] = o.transpose(0, 2, 3, 1)
    return full


# revision 21
# speedup vs baseline: 1.0071x; 1.0071x over previous
"""Binary-weight 3x3 SAME conv + bias + ReLU for (16,224,224,64)x(3,3,64,128),
distributed over 8 Trainium2 NeuronCores.

Distribution: data-parallel over the batch (N=16 -> 2 images per core), conv
weights (tiny, binarized to +/-1) replicated on every core. Forward only, so
no collectives are needed.

Per-core kernel design:
  - Host prep: x is transposed to NCHW and zero-padded by a 1-pixel halo into
    an fp16 tensor xp[2, 64, 226, 226]; W is binarized with sign() and packed
    per-tap into an fp16 [128, 9*128] lhsT block (rows 0-63 and 64-127 are
    identical copies, one per image slot).
  - SBUF layout puts input channels on partitions: image 0 occupies
    partitions 0-63, image 1 partitions 64-127. The two images' K=64 matmuls
    auto-derive PE tile_position (0,0)/(64,0) and run concurrently on
    disjoint row-groups of the 128x128 PE array - full array utilization
    without stacking copies.
  - A 3x3 conv is 9 shifted matmuls accumulated in one PSUM bank. The input
    strip is flattened [c, h*226]; a +/-1 column shift is a free-dim AP
    offset, a row shift is +/-226. Each PSUM tile covers two padded output
    rows (N = 2*226-2 = 450 <= 512 fp32 bank limit); the pad columns between
    the rows compute garbage that is simply never drained. Measured steady
    duo cadence is N/2.4GHz + 2.5ns = ~190ns, 98.7% of PE peak.
  - Drain = bias-add + ReLU straight out of PSUM, alternating between
    ScalarE (activation Relu with per-partition bias) and VectorE
    (tensor_scalar add/max) to split the elementwise load, writing dense
    fp32 [cout, 16*224] staging tiles that DMA contiguously to HBM.
  - fp16 activations keep the absmax error ~2e-4 relative to the fp32
    reference (weights are exactly +/-1 in fp16, accumulation is fp32 PSUM).
"""

import os
import sys

import numpy as np

for _p in ("/opt/trn_rl_repo", "/root/.axon_site/_ro/trn_rl_repo", "/root/.axon_site"):
    if os.path.isdir(_p) and _p not in sys.path:
        sys.path.append(_p)

import concourse.bass as bass
import concourse.mybir as mybir
import concourse.tile as tile
from concourse import bacc
from concourse.bass_utils import run_bass_kernel_spmd

# Problem shape (hardcoded per contract).
N_FULL, H, W_, CIN, COUT = 16, 224, 224, 64, 128
N_CORES = 8
IMGS = N_FULL // N_CORES  # images per core


def build_conv_program(imgs=IMGS, h=H, w=W_, cin=CIN, cout=COUT, strip_out=16):
    """Build the per-core Bass program. Parameterized so a scaled-down
    variant can be validated in CoreSim."""
    assert imgs == 2 and cin == 64 and cout == 128
    assert h % strip_out == 0 and strip_out % 2 == 0
    hp, wp = h + 2, w + 2
    n_strips = h // strip_out
    pairs = strip_out // 2  # output-row pairs per strip
    rows = strip_out + 2  # input rows needed per strip
    nfree = 2 * wp - 2  # matmul free dim (2 padded rows, minus trailing pads)
    npsum = 2 * wp  # PSUM tile width (trailing 2 cols never written/read)
    assert nfree <= 512
    f16, f32 = mybir.dt.float16, mybir.dt.float32

    nc = bacc.Bacc("TRN2", target_bir_lowering=False, debug=False)
    xp = nc.dram_tensor("xp", [imgs, cin, hp, wp], f16, kind="ExternalInput")
    wq = nc.dram_tensor("wq", [2 * cin, 9 * cout], f16, kind="ExternalInput")
    bias = nc.dram_tensor("bias", [cout, 1], f32, kind="ExternalInput")
    out = nc.dram_tensor("out", [imgs, cout, h, w], f32, kind="ExternalOutput")

    with tile.TileContext(nc) as tc:
        with (
            tc.tile_pool(name="const", bufs=1) as cpool,
            tc.tile_pool(name="xin", bufs=2) as xpool,
            tc.tile_pool(name="ps", bufs=8, space="PSUM") as pspool,
            tc.tile_pool(name="ostage", bufs=4) as opool,
        ):
            wsb = cpool.tile([128, 9 * cout], f16)
            nc.sync.dma_start(out=wsb[:], in_=wq[:])
            bsb = cpool.tile([cout, 1], f32)
            nc.sync.dma_start(out=bsb[:], in_=bias[:])
            warm = cpool.tile([cout, 1], f32)
            # PE warm-up: a dense ~9us accumulation group of dummy matmuls
    # on memset-zero SBUF (no DMA dependency) trips the HAM busy
            # window (~3.4us) early, so the real stream starts at 2.4GHz
            # instead of running its first ~2 strips at 1.2GHz. A single
            # accumulation group is essential: independent start/stop
            # matmuls on one tile get WAW-serialized by semaphores and
            # the resulting gappy burst never un-throttles the clock.
            dummy_x = cpool.tile([128, 512], f16)
            nc.vector.memset(dummy_x[:], 0)
            dummy_ps = pspool.tile([cout, 512], f32, name="dummy_ps", tag="pt")
            n_warm = 16
            for k in range(n_warm):
                nc.tensor.matmul(
                    dummy_ps[:],
                    dummy_x[:, 0:cout],
                    dummy_x[:, 0:512],
                    start=(k == 0),
                    stop=(k == n_warm - 1),
                )

            for s in range(n_strips):
                r0 = s * strip_out
                # Input strip: both images' channels stacked on partitions;
                # free dim = flattened padded rows r0 .. r0+rows-1.
                xt = xpool.tile([128, rows * wp], f16)
                # Half-strip loads (on the scalar engine's HWDGE queue,
                # away from bulk output traffic on sync) so the first duos
                # of a strip unblock after ~half the load time.
                rh0 = rows // 2
                for ra, rb in [(0, rh0), (rh0, rows)]:
                    nc.scalar.dma_start(
                        out=xt[:, ra * wp : rb * wp],
                        in_=xp[:][:, :, r0 + ra : r0 + rb, :].rearrange(
                            "i c r q -> (i c) (r q)"
                        ),
                    )
                if s == 0:
                    # Warm the ACT Relu spline table (~2.7us one-time load,
                    # unmodeled by the scheduler) AFTER the first strip's
                    # load triggers - ahead of them on the scalar stream it
                    # stalls the whole input pipe behind the table DMA.
                    nc.scalar.activation(
                        warm[:], bsb[:], mybir.ActivationFunctionType.Relu, bias=0.0
                    )

                ots = [
                    opool.tile([cout, strip_out * w], f32, name=f"ot{i}", tag=f"ot{i}")
                    for i in range(imgs)
                ]

                # Per-duo PSUM flow: 2 banks allocated per output-row pair,
                # 8-slot pool lets ~4 duos pipeline; drains release slots in
                # issue order so the matmul stream never hits a block-sized
                # PSUM barrier.
                for d in range(pairs):
                    pt = [
                        pspool.tile([cout, npsum], f32, name=f"pt_{s}_{d}_{i}", tag="pt")
                        for i in range(imgs)
                    ]
                    # 9 shifted taps accumulate; emission order (t, i)
                    # alternates images so adjacent PE matmuls land on
                    # disjoint row-groups and run concurrently.
                    for t in range(9):
                        dy, dx = divmod(t, 3)
                        base = (2 * d + dy) * wp + dx
                        for i in range(imgs):
                            nc.tensor.matmul(
                                pt[i][:, :nfree],
                                wsb[i * cin : (i + 1) * cin, t * cout : (t + 1) * cout],
                                xt[i * cin : (i + 1) * cin, base : base + nfree],
                                start=(t == 0),
                                stop=(t == 8),
                            )
                    # Drain: bias + ReLU, skipping the 2 pad columns per row.
                    for i in range(imgs):
                        src = pt[i][:].rearrange("p (r q) -> p r q", q=wp)[:, :, :w]
                        dst = ots[i][:, d * 2 * w : (d + 1) * 2 * w].rearrange(
                            "p (r q) -> p r q", q=w
                        )
                        if (d + i) % 2 == 0:
                            nc.scalar.activation(
                                dst,
                                src,
                                mybir.ActivationFunctionType.Relu,
                                bias=bsb[:, 0:1],
                            )
                        else:
                            nc.vector.tensor_scalar(
                                dst,
                                src,
                                bsb[:, 0:1],
                                0.0,
                                mybir.AluOpType.add,
                                mybir.AluOpType.max,
                            )
                    # Half-strip output DMA on the otherwise-idle sync
                    # engine; quarter-strip on the final strip so the
                    # kernel-tail transfer is short. (Finer everywhere
                    # regresses: the extra DMA-completion stalls break the
                    # PE busy window and re-throttle the HAM clock.)
                    qg = max(1, pairs // (4 if s == n_strips - 1 else 2))
                    if d % qg == qg - 1:
                        quart = d // qg
                        rh = 2 * qg
                        for i in range(imgs):
                            nc.sync.dma_start(
                                out=out[:][
                                    i, :, r0 + quart * rh : r0 + (quart + 1) * rh, :
                                ].rearrange("c r q -> c (r q)"),
                                in_=ots[i][:, quart * rh * w : (quart + 1) * rh * w],
                            )

    nc.compile()
    return nc


def prep_inputs(x, W, b, imgs=IMGS, h=H, w=W_, cin=CIN, cout=COUT, n_cores=N_CORES):
    """Host-side shard + layout prep. Returns per-core input maps."""
    hp, wp = h + 2, w + 2
    n = imgs * n_cores
    # Binarize weights; pack per-tap lhsT blocks, duplicated per image slot.
    wq_np = np.sign(np.asarray(W, dtype=np.float32)).astype(np.float16)
    wq_host = np.empty((2 * cin, 9 * cout), np.float16)
    for t in range(9):
        dy, dx = divmod(t, 3)
        wq_host[0:cin, t * cout : (t + 1) * cout] = wq_np[dy, dx]
        wq_host[cin : 2 * cin, t * cout : (t + 1) * cout] = wq_np[dy, dx]
    bias_host = np.ascontiguousarray(
        np.asarray(b, dtype=np.float32).reshape(cout, 1)
    )
    # NHWC -> NCHW, fp16, 1-pixel zero halo.
    xp_host = np.zeros((n, cin, hp, wp), np.float16)
    xp_host[:, :, 1 : h + 1, 1 : w + 1] = np.asarray(x).transpose(0, 3, 1, 2)
    return [
        {
            "xp": np.ascontiguousarray(xp_host[c * imgs : (c + 1) * imgs]),
            "wq": wq_host,
            "bias": bias_host,
        }
        for c in range(n_cores)
    ]


_NC_CACHE = {}


def _get_program():
    if "nc" not in _NC_CACHE:
        _NC_CACHE["nc"] = build_conv_program()
    return _NC_CACHE["nc"]


def kernel(x, W, b):
    """Full-input entry point: x (16,224,224,64) f32 NHWC, W (3,3,64,128) f32
    HWIO, b (128,) f32 -> (16,224,224,128) f32 NHWC."""
    nc = _get_program()
    in_maps = prep_inputs(x, W, b)
    res = run_bass_kernel_spmd(nc, in_maps, core_ids=list(range(N_CORES)))
    # Gather: per-core [2, 128, 224, 224] -> full NHWC.
    full = np.empty((N_FULL, H, W_, COUT), np.float32)
    for c in range(N_CORES):
        o = res.results[c]["out"]
        full[c * IMGS : (c + 1) * IMGS] = o.transpose(0, 2, 3, 1)
    return full
